# revision 18
# baseline (speedup 1.0000x reference)
"""Trainium2 Bass kernel for a seq2seq decoder step:
embedding lookup + Bahdanau attention + GRU cell + vocab projection.

Sharding: data-parallel over batch (32/core) for attention+GRU;
vocab-parallel (4000/core) for the fc projection, with an on-device
AllGather of h_new between the two parts.

All heavy matmuls run in bf16 (fp32 PSUM accumulation). Activations are
kept "feature-major" ([feature, batch]) where that lets the 128-wide
partition dim stay full; the GRU elementwise runs batch-major [32, .].

v2 layout: streaming pools are allocated up front so their DMAs prefetch
under the proj_e compute phase; GRU's hidden/embedded matmuls run early;
biases are folded in as rank-1 (K=1) matmuls so tanh runs as one big
ScalarE op per tile.
"""

import os
import sys

import numpy as np

sys.path.insert(0, "/opt/trn_rl_repo")

import ml_dtypes  # noqa: E402

import concourse.bacc as bacc  # noqa: E402
import concourse.bass as bass  # noqa: E402
import concourse.mybir as mybir  # noqa: E402
import concourse.tile as tile  # noqa: E402
from concourse.bass_utils import run_bass_kernel_spmd  # noqa: E402
from concourse.masks import make_identity  # noqa: E402

BF16 = mybir.dt.bfloat16
F32 = mybir.dt.float32
NPBF16 = ml_dtypes.bfloat16

# problem shapes (hardcoded per spec nn_Decoder_85650237817493)
V, M, E2, D = 32000, 512, 1024, 1024
B, S = 256, 128
NCORES = 8
BC = B // NCORES          # 32  local batch
VC = V // NCORES          # 4000 local vocab slice
G3 = 3 * D                # 3072 GRU gates
FI = M + E2               # 1536 rnn_in features
BS = BC * S               # 4096 local (batch, s) pairs
DCH = D // 128            # 8    feature chunks of 128
ECH = E2 // 128           # 8
MCH = M // 128            # 4
FCH = FI // 128           # 12
QW = 512                  # proj_e column tile (bs dim)
NQ = BS // QW             # 8
BPQ = QW // S             # 4 batches per column tile
VQ = 500                  # fc vocab tile (one PSUM bank)
NVQ = VC // VQ            # 8


def _emit(nc: bass.Bass, tc: "tile.TileContext", io: dict) -> None:
    from contextlib import ExitStack

    AF = mybir.ActivationFunctionType
    AL = mybir.AluOpType

    with ExitStack() as ctx:
        # ---------------- persistent pool: constants + small tiles --------
        singles = ctx.enter_context(tc.tile_pool(name="singles", bufs=1))
        ident_bf = singles.tile([128, 128], BF16)
        ident_f32 = singles.tile([128, 128], F32)
        make_identity(nc, ident_bf[:])
        make_identity(nc, ident_f32[:])
        ones_col = singles.tile([128, 1], BF16)
        nc.vector.memset(ones_col[:], 1.0)
        ones_row = singles.tile([1, 128], BF16)
        nc.vector.memset(ones_row[:], 1.0)


        hid_t_sb = singles.tile([128, DCH * BC], BF16)      # hidden^T
        nc.sync.dma_start(
            out=hid_t_sb[:].rearrange("p (c b) -> p c b", c=DCH),
            in_=io["hid_t"].ap().rearrange("(c p) b -> p c b", p=128))
        emb_sb = singles.tile([128, MCH * BC], BF16)        # embedded^T
        nc.sync.dma_start(
            out=emb_sb[:].rearrange("p (c b) -> p c b", c=MCH),
            in_=io["emb_t"].ap().rearrange("(c p) b -> p c b", p=128))
        v_sb = singles.tile([128, DCH], BF16)
        nc.sync.dma_start(
            out=v_sb[:], in_=io["v_w"].ap().rearrange("(c p) -> p c", p=128))
        bab_row = singles.tile([1, D], BF16)                # ba + bUa (row)
        nc.sync.dma_start(out=bab_row[:], in_=io["bab"].ap())
        hid_f_sb = singles.tile([BC, D], F32)
        nc.sync.dma_start(out=hid_f_sb[:], in_=io["hid_f"].ap())

        ph_sb = singles.tile([128, DCH * BC], BF16)         # proj_h^T + bias
        exp_sb = singles.tile([1, BS], BF16)
        expT_sb = singles.tile([128, BC], BF16)
        recip_row = singles.tile([1, BC], F32)
        recip_col = singles.tile([BC, 1], F32)
        attn_sb = singles.tile([BC, S], F32)
        ctx_bf = singles.tile([BC, E2], BF16)
        ctx_t_sb = singles.tile([128, ECH * BC], BF16)
        hnt_sb = singles.tile([128, DCH * BC], BF16)
        hnew_sb = singles.tile([BC, D], F32)
        gh_all = singles.tile([BC, G3], BF16)               # hid @ W_hh^T + b_hh
        gie_all = singles.tile([BC, G3], BF16)              # emb part of gi + b_ih
        ghe_rz = singles.tile([BC, 2 * D], BF16)            # gh_rz + gie_rz
        hna_sb = singles.tile([128, DCH * B], BF16)         # gathered h_new^T

        # ---------------- streaming pools (DMAs prefetch under compute) ---
        wg_pool = ctx.enter_context(tc.tile_pool(name="wg_pool", bufs=5))
        encn_pool = ctx.enter_context(tc.tile_pool(name="encn_pool", bufs=8))
        encq_pool = ctx.enter_context(tc.tile_pool(name="encq_pool", bufs=2))
        energy_pool = ctx.enter_context(tc.tile_pool(name="energy_pool", bufs=2))
        upool = ctx.enter_context(tc.tile_pool(name="upool", bufs=1))
        ua_sb = upool.tile([128, ECH * D], BF16)            # Ua^T resident
        for ec in range(ECH):
            nc.sync.dma_start(
                out=ua_sb[:, ec * D:(ec + 1) * D],
                in_=io["ua_t"].ap()[ec * 128:(ec + 1) * 128, :])
        fcw_pool = ctx.enter_context(tc.tile_pool(name="fcw_pool", bufs=4))
        bias_pool = ctx.enter_context(tc.tile_pool(name="bias_pool", bufs=1))
        bih_sb = bias_pool.tile([1, G3], BF16)
        nc.sync.dma_start(out=bih_sb[:], in_=io["bih"].ap())
        bhh_sb = bias_pool.tile([1, G3], BF16)
        nc.sync.dma_start(out=bhh_sb[:], in_=io["bhh"].ap())
        fcb_sb = bias_pool.tile([1, VC], BF16)
        nc.sync.dma_start(out=fcb_sb[:], in_=io["fcb"].ap())

        # prefetch the first fc weight blocks now; the rest stream in the
        # fc loop.  block vq = [128, (dc, 500)] of fc_W^T columns.
        fcw_tiles = {}

        def load_fcw(vq):
            t = fcw_pool.tile([128, DCH * VQ], BF16, tag="fcw", name="fcw_t")
            for dc in range(DCH):
                nc.sync.dma_start(
                    out=t[:, dc * VQ:(dc + 1) * VQ],
                    in_=io["fcw_t"].ap()[dc * 128:(dc + 1) * 128,
                                         vq * VQ:(vq + 1) * VQ])
            fcw_tiles[vq] = t

        for vq in range(4):
            load_fcw(vq)

        # prefetch enc_n tiles (used by the context matvecs much later)
        encn_tiles = {}

        def load_encn(b):
            t = encn_pool.tile([128, E2], BF16, tag="encn", name="encn_t")
            nc.sync.dma_start(
                out=t[:], in_=io["enc_n"].ap()[b * S:(b + 1) * S, :])
            encn_tiles[b] = t

        for b in range(6):
            load_encn(b)

        # ---------------- proj_h (batch-major) + bias ----------------------
        with ExitStack() as wctx:
            wpool = wctx.enter_context(tc.tile_pool(name="wpool", bufs=1))
            wa_sb = wpool.tile([128, ECH * D], BF16)        # Wa^T
            for ec in range(ECH):
                nc.sync.dma_start(
                    out=wa_sb[:, ec * D:(ec + 1) * D],
                    in_=io["wa_t"].ap()[ec * 128:(ec + 1) * 128, :])
            ph_psum = wctx.enter_context(
                tc.tile_pool(name="ph_psum", bufs=2, space="PSUM"))
            ph_t = ph_psum.tile([BC, D], F32, tag="pht", name="ph_t")
            for ec in range(ECH):
                lhs = hid_t_sb[:, ec * BC:(ec + 1) * BC]
                for j in range(D // 512):
                    nc.tensor.matmul(
                        ph_t[:BC, j * 512:(j + 1) * 512], lhs,
                        wa_sb[:, ec * D + j * 512: ec * D + (j + 1) * 512],
                        start=(ec == 0), stop=False)
            for j in range(D // 512):
                nc.tensor.matmul(
                    ph_t[:BC, j * 512:(j + 1) * 512], ones_row[:1, :BC],
                    bab_row[:1, j * 512:(j + 1) * 512],
                    start=False, stop=True)
            ph_row = wpool.tile([BC, D], BF16)
            nc.scalar.activation(ph_row[:], ph_t[:BC, :], AF.Copy)
            # transpose to d-major for the per-batch tanh bias
            for dc in range(DCH):
                pt_t = ph_psum.tile([128, BC], BF16, tag="ptt", name="pt_t")
                nc.tensor.matmul(
                    pt_t[:], ph_row[:, dc * 128:(dc + 1) * 128],
                    ident_bf[:BC, :BC], is_transpose=True)
                nc.vector.tensor_copy(ph_sb[:, dc * BC:(dc + 1) * BC],
                                      pt_t[:])

        # ---------------- proj_e + tanh + scores + early GRU --------------
        with ExitStack() as pctx:
            pe_psum = pctx.enter_context(
                tc.tile_pool(name="pe_psum", bufs=2, space="PSUM"))
            sc_psum = pctx.enter_context(
                tc.tile_pool(name="sc_psum", bufs=2, space="PSUM"))

            def proj_e_quarter(q):
                encq = encq_pool.tile([128, ECH * QW], BF16, tag="encq",
                                      name="encq_t")
                for ec in range(ECH):
                    nc.sync.dma_start(
                        out=encq[:, ec * QW:(ec + 1) * QW],
                        in_=io["enc_t"].ap()[ec * 128:(ec + 1) * 128,
                                             q * QW:(q + 1) * QW])
                sc_t = sc_psum.tile([1, QW], F32, tag="sc", name="sc_t")
                for dc in range(DCH):
                    pe_t = pe_psum.tile([128, QW], F32, tag="pe", name="pe_t")
                    for ec in range(ECH):
                        nc.tensor.matmul(
                            pe_t[:],
                            ua_sb[:, ec * D + dc * 128: ec * D + (dc + 1) * 128],
                            encq[:, ec * QW:(ec + 1) * QW],
                            start=(ec == 0), stop=(ec == ECH - 1))
                    en_t = energy_pool.tile([128, QW], BF16, tag="en",
                                            name="en_t")
                    for bb in range(BPQ):
                        b = q * BPQ + bb
                        nc.scalar.activation(
                            en_t[:, bb * S:(bb + 1) * S],
                            pe_t[:, bb * S:(bb + 1) * S],
                            AF.Tanh,
                            bias=ph_sb[:, dc * BC + b: dc * BC + b + 1])
                    nc.tensor.matmul(
                        sc_t[:1, :], v_sb[:, dc:dc + 1], en_t[:],
                        start=(dc == 0), stop=(dc == DCH - 1))
                nc.scalar.activation(
                    exp_sb[:1, q * QW:(q + 1) * QW], sc_t[:1, :], AF.Exp)

            proj_e_quarter(0)

            # early GRU matmuls (hidden/embedded parts) while enc streams
            with ExitStack() as gctx:
                gh_psum = gctx.enter_context(
                    tc.tile_pool(name="gh_psum", bufs=1, space="PSUM"))
                for half in range(2):
                    gsl = slice(half * (G3 // 2), (half + 1) * (G3 // 2))
                    gh_t = gh_psum.tile([BC, G3 // 2], F32, tag="gh",
                                        name="gh_t")
                    for c in range(DCH):
                        w_t = wg_pool.tile([128, G3], BF16, tag="wg",
                                           name="whh_t")
                        nc.sync.dma_start(
                            out=w_t[:],
                            in_=io["whh_t"].ap()[c * 128:(c + 1) * 128, :])
                        lhs = hid_t_sb[:, c * BC:(c + 1) * BC]
                        for j in range(G3 // 2 // 512):
                            nc.tensor.matmul(
                                gh_t[:BC, j * 512:(j + 1) * 512], lhs,
                                w_t[:, half * (G3 // 2) + j * 512:
                                    half * (G3 // 2) + (j + 1) * 512],
                                start=(c == 0), stop=False)
                    for j in range(G3 // 2 // 512):
                        nc.tensor.matmul(
                            gh_t[:BC, j * 512:(j + 1) * 512],
                            ones_row[:1, :BC],
                            bhh_sb[:1, half * (G3 // 2) + j * 512:
                                   half * (G3 // 2) + (j + 1) * 512],
                            start=False, stop=(j == G3 // 2 // 512 - 1))
                    nc.scalar.activation(gh_all[:, gsl], gh_t[:BC, :],
                                         AF.Copy)

            proj_e_quarter(1)

            with ExitStack() as gctx:
                gi_psum = gctx.enter_context(
                    tc.tile_pool(name="gi_psum", bufs=1, space="PSUM"))
                for half in range(2):
                    gsl = slice(half * (G3 // 2), (half + 1) * (G3 // 2))
                    gi_t = gi_psum.tile([BC, G3 // 2], F32, tag="gie",
                                        name="gi_t")
                    for c in range(MCH):
                        w_t = wg_pool.tile([128, G3], BF16, tag="wg",
                                           name="wih_t")
                        nc.sync.dma_start(
                            out=w_t[:],
                            in_=io["wih_t"].ap()[c * 128:(c + 1) * 128, :])
                        lhs = emb_sb[:, c * BC:(c + 1) * BC]
                        for j in range(G3 // 2 // 512):
                            nc.tensor.matmul(
                                gi_t[:BC, j * 512:(j + 1) * 512], lhs,
                                w_t[:, half * (G3 // 2) + j * 512:
                                    half * (G3 // 2) + (j + 1) * 512],
                                start=(c == 0), stop=False)
                    for j in range(G3 // 2 // 512):
                        nc.tensor.matmul(
                            gi_t[:BC, j * 512:(j + 1) * 512],
                            ones_row[:1, :BC],
                            bih_sb[:1, half * (G3 // 2) + j * 512:
                                   half * (G3 // 2) + (j + 1) * 512],
                            start=False, stop=(j == G3 // 2 // 512 - 1))
                    nc.scalar.activation(gie_all[:, gsl], gi_t[:BC, :],
                                         AF.Copy)
            # pre-combine r,z parts off the critical path
            nc.vector.tensor_tensor(ghe_rz[:], gh_all[:, :2 * D],
                                    gie_all[:, :2 * D], AL.add)

            for q in range(2, NQ):
                proj_e_quarter(q)

        # ---------------- softmax + context -------------------------------
        with ExitStack() as cctx:
            tp_psum = cctx.enter_context(
                tc.tile_pool(name="tp_psum", bufs=4, space="PSUM"))
            for b in range(BC):
                et = tp_psum.tile([128, 1], BF16, tag="tp", name="et")
                nc.tensor.matmul(
                    et[:], exp_sb[:1, b * S:(b + 1) * S], ident_bf[:1, :1],
                    is_transpose=True)
                nc.vector.tensor_copy(expT_sb[:, b:b + 1], et[:])
            sum_t = tp_psum.tile([1, BC], F32, tag="tp", name="sum_t")
            nc.tensor.matmul(sum_t[:1, :], ones_col[:], expT_sb[:])
            nc.vector.reciprocal(recip_row[:1, :], sum_t[:1, :])
            rc_t = tp_psum.tile([BC, 1], F32, tag="tp", name="rc_t")
            nc.tensor.matmul(rc_t[:], recip_row[:1, :], ident_f32[:1, :1],
                             is_transpose=True)
            nc.vector.tensor_copy(recip_col[:], rc_t[:])
            at_t = tp_psum.tile([BC, S], BF16, tag="tp", name="at_t")
            nc.tensor.matmul(at_t[:], expT_sb[:], ident_bf[:, :],
                             is_transpose=True)
            nc.vector.tensor_scalar_mul(attn_sb[:], at_t[:], recip_col[:])
            nc.sync.dma_start(out=io["attn_out"].ap(), in_=attn_sb[:])

            # context: per-batch matvec -> part-0 rows -> DRAM -> [32, e]
            ctx_psum = cctx.enter_context(
                tc.tile_pool(name="ctx_psum", bufs=2, space="PSUM"))
            cpool = cctx.enter_context(tc.tile_pool(name="cpool", bufs=2))
            cdram = cctx.enter_context(
                tc.tile_pool(name="cdram", bufs=1, space="DRAM"))
            ctx_dram = cdram.tile([BC, E2], BF16)
            GRP = 4
            for g in range(BC // GRP):
                cflat = cpool.tile([1, GRP * E2], BF16, tag="cflat",
                                   name="cflat")
                for i in range(GRP):
                    b = g * GRP + i
                    if b not in encn_tiles:
                        load_encn(b)
                    encn_t = encn_tiles.pop(b)
                    cx_t = ctx_psum.tile([1, E2], F32, tag="cx", name="cx_t")
                    for j in range(E2 // 512):
                        nc.tensor.matmul(
                            cx_t[:1, j * 512:(j + 1) * 512],
                            expT_sb[:, b:b + 1],
                            encn_t[:, j * 512:(j + 1) * 512])
                    dst = cflat[:1, i * E2:(i + 1) * E2]
                    if b % 2 == 0:
                        nc.scalar.activation(dst, cx_t[:1, :], AF.Copy,
                                             scale=recip_row[:1, b:b + 1])
                    else:
                        nc.vector.tensor_scalar_mul(dst, cx_t[:1, :],
                                                    recip_row[:1, b:b + 1])
                nc.sync.dma_start(
                    out=ctx_dram[g * GRP:(g + 1) * GRP, :], in_=cflat[:1, :])
            nc.sync.dma_start(out=ctx_bf[:], in_=ctx_dram[:])
            for ec in range(ECH):
                ct_t = tp_psum.tile([128, BC], BF16, tag="tp", name="ct_t")
                nc.tensor.matmul(
                    ct_t[:], ctx_bf[:, ec * 128:(ec + 1) * 128],
                    ident_bf[:BC, :BC], is_transpose=True)
                nc.vector.tensor_copy(ctx_t_sb[:, ec * BC:(ec + 1) * BC],
                                      ct_t[:])

        # ---------------- GRU tail ----------------------------------------
        with ExitStack() as gctx:
            gic_psum = gctx.enter_context(
                tc.tile_pool(name="gic_psum", bufs=1, space="PSUM"))
            gru = gctx.enter_context(tc.tile_pool(name="gru", bufs=1))
            gic_t = gic_psum.tile([BC, G3], F32)
            for c in range(ECH):
                w_t = wg_pool.tile([128, G3], BF16, tag="wg", name="wih2_t")
                nc.sync.dma_start(
                    out=w_t[:],
                    in_=io["wih_t"].ap()[(MCH + c) * 128:
                                         (MCH + c + 1) * 128, :])
                lhs = ctx_t_sb[:, c * BC:(c + 1) * BC]
                for j in range(G3 // 512):
                    nc.tensor.matmul(
                        gic_t[:BC, j * 512:(j + 1) * 512], lhs,
                        w_t[:, j * 512:(j + 1) * 512],
                        start=(c == 0), stop=(c == ECH - 1))
            t_rz = gru.tile([BC, 2 * D], F32)
            r_sb = gru.tile([BC, D], F32)
            z_sb = gru.tile([BC, D], F32)
            n_sb = gru.tile([BC, D], F32)
            tmp = gru.tile([BC, D], F32)
            nc.vector.tensor_tensor(t_rz[:], gic_t[:BC, :2 * D], ghe_rz[:],
                                    AL.add)
            nc.scalar.activation(r_sb[:], t_rz[:, :D], AF.Sigmoid)
            nc.scalar.activation(z_sb[:], t_rz[:, D:], AF.Sigmoid)
            nc.vector.tensor_tensor(tmp[:], gic_t[:BC, 2 * D:],
                                    gie_all[:, 2 * D:], AL.add)
            nc.vector.tensor_tensor(n_sb[:], r_sb[:], gh_all[:, 2 * D:],
                                    AL.mult)
            nc.vector.tensor_tensor(tmp[:], tmp[:], n_sb[:], AL.add)
            nc.scalar.activation(n_sb[:], tmp[:], AF.Tanh)
            nc.vector.tensor_tensor(tmp[:], hid_f_sb[:], n_sb[:], AL.subtract)
            nc.vector.tensor_tensor(tmp[:], z_sb[:], tmp[:], AL.mult)
            nc.vector.tensor_tensor(hnew_sb[:], n_sb[:], tmp[:], AL.add)
            nc.sync.dma_start(out=io["hnew_out"].ap(), in_=hnew_sb[:])

        # ---------------- allgather + fc ----------------------------------
        with ExitStack() as dctx:
            ht_psum = dctx.enter_context(
                tc.tile_pool(name="ht_psum", bufs=2, space="PSUM"))
            for ec in range(DCH):
                ht_t = ht_psum.tile([128, BC], F32, tag="ht", name="ht_t")
                nc.tensor.matmul(
                    ht_t[:], hnew_sb[:, ec * 128:(ec + 1) * 128],
                    ident_f32[:BC, :BC], is_transpose=True)
                nc.vector.tensor_copy(hnt_sb[:, ec * BC:(ec + 1) * BC],
                                      ht_t[:])
            dram = dctx.enter_context(
                tc.tile_pool(name="dram", bufs=1, space="DRAM"))
            hn_loc = dram.tile([D, BC], BF16)
            hn_all = dram.tile([NCORES * D, BC], BF16, addr_space="Shared")
            nc.sync.dma_start(
                out=hn_loc[:].rearrange("(c p) b -> p c b", p=128),
                in_=hnt_sb[:].rearrange("p (c b) -> p c b", c=DCH))
            nc.gpsimd.collective_compute(
                "AllGather", mybir.AluOpType.bypass,
                replica_groups=[list(range(NCORES))],
                ins=[hn_loc[:].opt()], outs=[hn_all[:].opt()])
            for dc in range(DCH):
                nc.sync.dma_start(
                    out=hna_sb[:, dc * B:(dc + 1) * B].rearrange(
                        "p (c b) -> p c b", c=NCORES),
                    in_=hn_all[:].rearrange(
                        "(c k p) b -> k p c b", c=NCORES, p=128)[dc])

            fc_psum = dctx.enter_context(
                tc.tile_pool(name="fc_psum", bufs=4, space="PSUM"))
            pred_pool = dctx.enter_context(
                tc.tile_pool(name="pred_pool", bufs=3))
            for vq in range(NVQ):
                if vq not in fcw_tiles:
                    load_fcw(vq)
                fq = fcw_tiles.pop(vq)
                for bt in range(B // 128):
                    fc_t = fc_psum.tile([128, VQ], F32, tag="fc", name="fc_t")
                    for dc in range(DCH):
                        nc.tensor.matmul(
                            fc_t[:],
                            hna_sb[:, dc * B + bt * 128:
                                   dc * B + (bt + 1) * 128],
                            fq[:, dc * VQ:(dc + 1) * VQ],
                            start=(dc == 0), stop=False)
                    nc.tensor.matmul(
                        fc_t[:], ones_row[:1, :],
                        fcb_sb[:1, vq * VQ:(vq + 1) * VQ],
                        start=False, stop=True)
                    pred_t = pred_pool.tile([128, VQ], F32, tag="pred",
                                            name="pred_t")
                    nc.scalar.activation(pred_t[:], fc_t[:], AF.Copy)
                    nc.sync.dma_start(
                        out=io["pred_out"].ap()[bt * 128:(bt + 1) * 128,
                                                vq * VQ:(vq + 1) * VQ],
                        in_=pred_t[:])


def _build_program() -> bass.Bass:
    nc = bacc.Bacc("TRN2", target_bir_lowering=False, debug=False,
                   num_devices=NCORES)
    io = {}
    io["emb_t"] = nc.dram_tensor("emb_t", [M, BC], BF16, kind="ExternalInput")
    io["hid_t"] = nc.dram_tensor("hid_t", [D, BC], BF16, kind="ExternalInput")
    io["hid_f"] = nc.dram_tensor("hid_f", [BC, D], F32, kind="ExternalInput")
    io["enc_t"] = nc.dram_tensor("enc_t", [E2, BS], BF16, kind="ExternalInput")
    io["enc_n"] = nc.dram_tensor("enc_n", [BS, E2], BF16, kind="ExternalInput")
    io["wa_t"] = nc.dram_tensor("wa_t", [D, D], BF16, kind="ExternalInput")
    io["ua_t"] = nc.dram_tensor("ua_t", [E2, D], BF16, kind="ExternalInput")
    io["bab"] = nc.dram_tensor("bab", [1, D], BF16, kind="ExternalInput")
    io["v_w"] = nc.dram_tensor("v_w", [D], BF16, kind="ExternalInput")
    io["wih_t"] = nc.dram_tensor("wih_t", [FI, G3], BF16, kind="ExternalInput")
    io["whh_t"] = nc.dram_tensor("whh_t", [D, G3], BF16, kind="ExternalInput")
    io["bih"] = nc.dram_tensor("bih", [1, G3], BF16, kind="ExternalInput")
    io["bhh"] = nc.dram_tensor("bhh", [1, G3], BF16, kind="ExternalInput")
    io["fcw_t"] = nc.dram_tensor("fcw_t", [D, VC], BF16, kind="ExternalInput")
    io["fcb"] = nc.dram_tensor("fcb", [1, VC], BF16, kind="ExternalInput")
    if os.environ.get("DECODER_KERNEL_DEBUG"):
        io["dbg_hn"] = nc.dram_tensor("dbg_hn", [NCORES * D, BC], BF16,
                                      kind="ExternalOutput")
    io["pred_out"] = nc.dram_tensor("pred_out", [B, VC], F32,
                                    kind="ExternalOutput")
    io["hnew_out"] = nc.dram_tensor("hnew_out", [BC, D], F32,
                                    kind="ExternalOutput")
    io["attn_out"] = nc.dram_tensor("attn_out", [BC, S], F32,
                                    kind="ExternalOutput")

    with tile.TileContext(nc) as tc:
        _emit(nc, tc, io)
    nc.compile()
    return nc


_PROGRAM_CACHE = None


def _get_program():
    global _PROGRAM_CACHE
    if _PROGRAM_CACHE is None:
        _PROGRAM_CACHE = _build_program()
    return _PROGRAM_CACHE


def _prep_in_maps(x, hidden, encoder_outputs, emb, Wa, ba, Ua, bUa, v,
                  W_ih, W_hh, b_ih, b_hh, fc_W, fc_b):
    f32 = np.float32
    x = np.asarray(x)
    hidden = np.asarray(hidden, f32)
    enc = np.asarray(encoder_outputs, f32)
    emb = np.asarray(emb, f32)
    embedded = emb[x]                              # [B, M] host gather
    bab = (np.asarray(ba, f32) + np.asarray(bUa, f32))
    bab = bab.astype(NPBF16).reshape(1, D)
    wa_t = np.ascontiguousarray(np.asarray(Wa, f32).T).astype(NPBF16)
    ua_t = np.ascontiguousarray(np.asarray(Ua, f32).T).astype(NPBF16)
    wih_t = np.ascontiguousarray(np.asarray(W_ih, f32).T).astype(NPBF16)
    whh_t = np.ascontiguousarray(np.asarray(W_hh, f32).T).astype(NPBF16)
    v_bf = np.asarray(v, f32).astype(NPBF16)
    bih = np.asarray(b_ih, f32).astype(NPBF16).reshape(1, G3)
    bhh = np.asarray(b_hh, f32).astype(NPBF16).reshape(1, G3)
    fc_W = np.asarray(fc_W, f32)
    fc_b = np.asarray(fc_b, f32)

    in_maps = []
    for c in range(NCORES):
        bs_ = slice(c * BC, (c + 1) * BC)
        vs_ = slice(c * VC, (c + 1) * VC)
        enc_c = enc[bs_]                                   # [32, 128, 1024]
        in_maps.append({
            "emb_t": np.ascontiguousarray(embedded[bs_].T).astype(NPBF16),
            "hid_t": np.ascontiguousarray(hidden[bs_].T).astype(NPBF16),
            "hid_f": np.ascontiguousarray(hidden[bs_]),
            "enc_t": np.ascontiguousarray(
                enc_c.transpose(2, 0, 1).reshape(E2, BS)).astype(NPBF16),
            "enc_n": np.ascontiguousarray(
                enc_c.reshape(BS, E2)).astype(NPBF16),
            "wa_t": wa_t,
            "ua_t": ua_t,
            "bab": bab,
            "v_w": v_bf,
            "wih_t": wih_t,
            "whh_t": whh_t,
            "bih": bih,
            "bhh": bhh,
            "fcw_t": np.ascontiguousarray(fc_W[vs_].T).astype(NPBF16),
            "fcb": fc_b[vs_].astype(NPBF16).reshape(1, VC),
        })
    return in_maps


def _install_ntff_hook():
    """The agent image's antenv lacks axon_hooks; synthesize it so
    trace=True can capture NTFF profiles through libaxon_pjrt.so."""
    import types

    if "antenv.axon_hooks" in sys.modules:
        return
    import antenv

    mod = types.ModuleType("antenv.axon_hooks")
    state = {"hook": None}
    mod.set_axon_ntff_profile_hook = lambda h: state.__setitem__("hook", h)
    mod.get_axon_ntff_profile_hook = lambda: state["hook"]
    sys.modules["antenv.axon_hooks"] = mod
    antenv.axon_hooks = mod
    try:
        from trn_agent_boot.trn_boot import _ntff_profile_via_ctypes
        mod.set_axon_ntff_profile_hook(
            _ntff_profile_via_ctypes("/opt/axon/libaxon_pjrt.so"))
    except Exception as e:  # degrade to no tracing
        print(f"ntff hook install failed: {e}", file=sys.stderr)
    # no S3 in this container; keep artifacts local
    import concourse.bass_utils as _bu
    _bu.upload_artifacts = lambda tmpdir: tmpdir


def _run(in_maps, trace=False):
    nc = _get_program()
    if trace:
        _install_ntff_hook()
    return run_bass_kernel_spmd(nc, in_maps, core_ids=list(range(NCORES)),
                                trace=trace)


def kernel(x, hidden, encoder_outputs, emb, Wa, ba, Ua, bUa, v,
           W_ih, W_hh, b_ih, b_hh, fc_W, fc_b, _trace=False,
           _want_results=False):
    in_maps = _prep_in_maps(x, hidden, encoder_outputs, emb, Wa, ba, Ua, bUa,
                            v, W_ih, W_hh, b_ih, b_hh, fc_W, fc_b)
    res = _run(in_maps, trace=_trace)
    pred = np.concatenate([res.results[c]["pred_out"] for c in range(NCORES)],
                          axis=1)
    h_new = np.concatenate([res.results[c]["hnew_out"] for c in range(NCORES)],
                           axis=0)
    attn_w = np.concatenate([res.results[c]["attn_out"] for c in range(NCORES)],
                            axis=0)
    out = (np.asarray(pred, np.float32), np.asarray(h_new, np.float32),
           np.asarray(attn_w, np.float32))
    if _want_results:
        return out, res
    return out


# revision 27
# speedup vs baseline: 1.0663x; 1.0663x over previous
"""Trainium2 Bass kernel for a seq2seq decoder step:
embedding lookup + Bahdanau attention + GRU cell + vocab projection.

Sharding: data-parallel over batch (32/core) for attention+GRU;
vocab-parallel (4000/core) for the fc projection, with an on-device
AllGather of h_new between the two parts.

All heavy matmuls run in bf16 (fp32 PSUM accumulation). Activations are
kept "feature-major" ([feature, batch]) where that lets the 128-wide
partition dim stay full; the GRU elementwise runs batch-major [32, .].

v2 layout: streaming pools are allocated up front so their DMAs prefetch
under the proj_e compute phase; GRU's hidden/embedded matmuls run early;
biases are folded in as rank-1 (K=1) matmuls so tanh runs as one big
ScalarE op per tile.
"""

import os
import sys

import numpy as np

sys.path.insert(0, "/opt/trn_rl_repo")

import ml_dtypes  # noqa: E402

import concourse.bacc as bacc  # noqa: E402
import concourse.bass as bass  # noqa: E402
import concourse.mybir as mybir  # noqa: E402
import concourse.tile as tile  # noqa: E402
from concourse.bass_utils import run_bass_kernel_spmd  # noqa: E402
from concourse.masks import make_identity  # noqa: E402

BF16 = mybir.dt.bfloat16
F32 = mybir.dt.float32
NPBF16 = ml_dtypes.bfloat16

# problem shapes (hardcoded per spec nn_Decoder_85650237817493)
V, M, E2, D = 32000, 512, 1024, 1024
B, S = 256, 128
NCORES = 8
BC = B // NCORES          # 32  local batch
VC = V // NCORES          # 4000 local vocab slice
G3 = 3 * D                # 3072 GRU gates
FI = M + E2               # 1536 rnn_in features
BS = BC * S               # 4096 local (batch, s) pairs
DCH = D // 128            # 8    feature chunks of 128
ECH = E2 // 128           # 8
MCH = M // 128            # 4
FCH = FI // 128           # 12
QW = 512                  # proj_e column tile (bs dim)
NQ = BS // QW             # 8
BPQ = QW // S             # 4 batches per column tile
VQ = 500                  # fc vocab tile (one PSUM bank)
NVQ = VC // VQ            # 8


def _emit(nc: bass.Bass, tc: "tile.TileContext", io: dict) -> None:
    from contextlib import ExitStack

    AF = mybir.ActivationFunctionType
    AL = mybir.AluOpType

    with ExitStack() as ctx:
        # ---------------- persistent pool: constants + small tiles --------
        singles = ctx.enter_context(tc.tile_pool(name="singles", bufs=1))
        ident_bf = singles.tile([128, 128], BF16)
        ident_f32 = singles.tile([128, 128], F32)
        make_identity(nc, ident_bf[:])
        make_identity(nc, ident_f32[:])
        ones_col = singles.tile([128, 1], BF16)
        nc.vector.memset(ones_col[:], 1.0)
        ones_row = singles.tile([1, 128], BF16)
        nc.vector.memset(ones_row[:], 1.0)


        hid_t_sb = singles.tile([128, DCH * BC], BF16)      # hidden^T
        nc.sync.dma_start(
            out=hid_t_sb[:].rearrange("p (c b) -> p c b", c=DCH),
            in_=io["hid_t"].ap().rearrange("(c p) b -> p c b", p=128))
        emb_sb = singles.tile([128, MCH * BC], BF16)        # embedded^T
        nc.sync.dma_start(
            out=emb_sb[:].rearrange("p (c b) -> p c b", c=MCH),
            in_=io["emb_t"].ap().rearrange("(c p) b -> p c b", p=128))
        v_sb = singles.tile([128, DCH], BF16)
        nc.sync.dma_start(
            out=v_sb[:], in_=io["v_w"].ap().rearrange("(c p) -> p c", p=128))
        bab_row = singles.tile([1, D], BF16)                # ba + bUa (row)
        nc.sync.dma_start(out=bab_row[:], in_=io["bab"].ap())
        hid_f_sb = singles.tile([BC, D], F32)
        nc.sync.dma_start(out=hid_f_sb[:], in_=io["hid_f"].ap())

        ph_sb = singles.tile([128, DCH * BC], BF16)         # proj_h^T + bias
        exp_sb = singles.tile([1, BS], BF16)
        expT_sb = singles.tile([128, BC], BF16)
        recip_row = singles.tile([1, BC], F32)
        recip_col = singles.tile([BC, 1], F32)
        attn_sb = singles.tile([BC, S], F32)
        ctx_bf = singles.tile([BC, E2], BF16)
        ctx_t_sb = singles.tile([128, ECH * BC], BF16)
        hnt_sb = singles.tile([128, DCH * BC], BF16)
        hnew_sb = singles.tile([BC, D], F32)
        gh_all = singles.tile([BC, G3], BF16)               # hid @ W_hh^T + b_hh
        gie_all = singles.tile([BC, G3], BF16)              # emb part of gi + b_ih
        ghe_rz = singles.tile([BC, 2 * D], BF16)            # gh_rz + gie_rz
        hna_sb = singles.tile([128, DCH * B], BF16)         # gathered h_new^T

        # ---------------- streaming pools (DMAs prefetch under compute) ---
        wg_pool = ctx.enter_context(tc.tile_pool(name="wg_pool", bufs=5))
        encq_pool = ctx.enter_context(tc.tile_pool(name="encq_pool", bufs=2))
        upool = ctx.enter_context(tc.tile_pool(name="upool", bufs=1))
        ua_sb = upool.tile([128, ECH * D], BF16)            # Ua^T resident
        for ec in range(ECH):
            nc.sync.dma_start(
                out=ua_sb[:, ec * D:(ec + 1) * D],
                in_=io["ua_t"].ap()[ec * 128:(ec + 1) * 128, :])
        fcw_pool = ctx.enter_context(tc.tile_pool(name="fcw_pool", bufs=5))
        cdram = ctx.enter_context(
            tc.tile_pool(name="cdram", bufs=1, space="DRAM"))
        ctx_dram = cdram.tile([BC, E2], BF16)
        bias_pool = ctx.enter_context(tc.tile_pool(name="bias_pool", bufs=1))
        bih_sb = bias_pool.tile([1, G3], BF16)
        nc.sync.dma_start(out=bih_sb[:], in_=io["bih"].ap())
        bhh_sb = bias_pool.tile([1, G3], BF16)
        nc.sync.dma_start(out=bhh_sb[:], in_=io["bhh"].ap())
        fcb_sb = bias_pool.tile([1, VC], BF16)
        nc.sync.dma_start(out=fcb_sb[:], in_=io["fcb"].ap())

        # fc weight blocks stream as [128, (dc, 500)] views of fc_W^T;
        # emission of their DMAs is interleaved into the main loop so the
        # critical-path loads (wa, enc, ua) go first in the queues.
        fcw_tiles = {}

        def load_fcw(vq):
            t = fcw_pool.tile([128, DCH * VQ], BF16, tag="fcw", name="fcw_t")
            for dc in range(DCH):
                nc.sync.dma_start(
                    out=t[:, dc * VQ:(dc + 1) * VQ],
                    in_=io["fcw_t"].ap()[dc * 128:(dc + 1) * 128,
                                         vq * VQ:(vq + 1) * VQ])
            fcw_tiles[vq] = t

        encn_tiles = {}

        # ---------------- proj_h (batch-major) + bias ----------------------
        with ExitStack() as wctx:
            wpool = wctx.enter_context(tc.tile_pool(name="wpool", bufs=1))
            wa_sb = wpool.tile([128, ECH * D], BF16)        # Wa^T
            for ec in range(ECH):
                nc.sync.dma_start(
                    out=wa_sb[:, ec * D:(ec + 1) * D],
                    in_=io["wa_t"].ap()[ec * 128:(ec + 1) * 128, :])
            ph_psum = wctx.enter_context(
                tc.tile_pool(name="ph_psum", bufs=2, space="PSUM"))
            ph_t = ph_psum.tile([BC, D], F32, tag="pht", name="ph_t")
            for ec in range(ECH):
                lhs = hid_t_sb[:, ec * BC:(ec + 1) * BC]
                for j in range(D // 512):
                    nc.tensor.matmul(
                        ph_t[:BC, j * 512:(j + 1) * 512], lhs,
                        wa_sb[:, ec * D + j * 512: ec * D + (j + 1) * 512],
                        start=(ec == 0), stop=False)
            for j in range(D // 512):
                nc.tensor.matmul(
                    ph_t[:BC, j * 512:(j + 1) * 512], ones_row[:1, :BC],
                    bab_row[:1, j * 512:(j + 1) * 512],
                    start=False, stop=True)
            ph_row = wpool.tile([BC, D], BF16)
            nc.scalar.activation(ph_row[:], ph_t[:BC, :], AF.Copy)
            # transpose to d-major for the per-batch tanh bias
            for dc in range(DCH):
                pt_t = ph_psum.tile([128, BC], BF16, tag="ptt", name="pt_t")
                nc.tensor.matmul(
                    pt_t[:], ph_row[:, dc * 128:(dc + 1) * 128],
                    ident_bf[:BC, :BC], is_transpose=True)
                nc.vector.tensor_copy(ph_sb[:, dc * BC:(dc + 1) * BC],
                                      pt_t[:])

        # ---------------- proj_e + tanh + scores + context + early GRU ----
        with ExitStack() as pctx:
            pe_psum = pctx.enter_context(
                tc.tile_pool(name="pe_psum", bufs=2, space="PSUM"))
            sc_psum = pctx.enter_context(
                tc.tile_pool(name="sc_psum", bufs=1, space="PSUM"))
            cpool = pctx.enter_context(tc.tile_pool(name="cpool", bufs=2))
            encn_pool = pctx.enter_context(
                tc.tile_pool(name="encn_pool", bufs=8))
            energy_pool = pctx.enter_context(
                tc.tile_pool(name="energy_pool", bufs=2))

            def load_encn(b):
                t = encn_pool.tile([128, E2], BF16, tag="encn", name="encn_t")
                nc.sync.dma_start(
                    out=t[:], in_=io["enc_n"].ap()[b * S:(b + 1) * S, :])
                encn_tiles[b] = t

            for b in range(BPQ):
                load_encn(b)

            def proj_e_quarter(q):
                encq = encq_pool.tile([128, ECH * QW], BF16, tag="encq",
                                      name="encq_t")
                for ec in range(ECH):
                    nc.sync.dma_start(
                        out=encq[:, ec * QW:(ec + 1) * QW],
                        in_=io["enc_t"].ap()[ec * 128:(ec + 1) * 128,
                                             q * QW:(q + 1) * QW])
                if q + 1 < NQ:
                    for bb in range(BPQ):
                        load_encn((q + 1) * BPQ + bb)
                sc_t = sc_psum.tile([1, QW], F32, tag="sc", name="sc_t")
                for dc in range(DCH):
                    pe_t = pe_psum.tile([128, QW], F32, tag="pe", name="pe_t")
                    for ec in range(ECH):
                        nc.tensor.matmul(
                            pe_t[:],
                            ua_sb[:, ec * D + dc * 128: ec * D + (dc + 1) * 128],
                            encq[:, ec * QW:(ec + 1) * QW],
                            start=(ec == 0), stop=(ec == ECH - 1))
                    en_t = energy_pool.tile([128, QW], BF16, tag="en",
                                            name="en_t")
                    for bb in range(BPQ):
                        b = q * BPQ + bb
                        nc.scalar.activation(
                            en_t[:, bb * S:(bb + 1) * S],
                            pe_t[:, bb * S:(bb + 1) * S],
                            AF.Tanh,
                            bias=ph_sb[:, dc * BC + b: dc * BC + b + 1])
                    nc.tensor.matmul(
                        sc_t[:1, :], v_sb[:, dc:dc + 1], en_t[:],
                        start=(dc == 0), stop=(dc == DCH - 1))
                nc.scalar.activation(
                    exp_sb[:1, q * QW:(q + 1) * QW], sc_t[:1, :], AF.Exp)
                # per-quarter softmax/context piece: transpose exp, then
                # UNNORMALIZED context matvecs (recip applied after q7)
                with ExitStack() as qctx:
                    q_psum = qctx.enter_context(
                        tc.tile_pool(name="q_psum", bufs=2, space="PSUM"))
                    cflat = cpool.tile([1, BPQ * E2], BF16, tag="cflat",
                                       name="cflat")
                    for bb in range(BPQ):
                        b = q * BPQ + bb
                        et = q_psum.tile([128, 1], BF16, tag="et", name="et",
                                         bufs=1)
                        nc.tensor.matmul(
                            et[:], exp_sb[:1, b * S:(b + 1) * S],
                            ident_bf[:1, :1], is_transpose=True)
                        nc.vector.tensor_copy(expT_sb[:, b:b + 1], et[:])
                        encn_t = encn_tiles.pop(b)
                        cx_t = q_psum.tile([1, E2], F32, tag="cx", name="cx_t")
                        for j in range(E2 // 512):
                            nc.tensor.matmul(
                                cx_t[:1, j * 512:(j + 1) * 512],
                                expT_sb[:, b:b + 1],
                                encn_t[:, j * 512:(j + 1) * 512])
                        dst = cflat[:1, bb * E2:(bb + 1) * E2]
                        if b % 2 == 0:
                            nc.scalar.activation(dst, cx_t[:1, :], AF.Copy)
                        else:
                            nc.vector.tensor_copy(dst, cx_t[:1, :])
                    nc.sync.dma_start(
                        out=ctx_dram[q * BPQ:(q + 1) * BPQ, :],
                        in_=cflat[:1, :])

            proj_e_quarter(0)

            # early GRU matmuls (hidden/embedded parts) while enc streams
            with ExitStack() as gctx:
                gh_psum = gctx.enter_context(
                    tc.tile_pool(name="gh_psum", bufs=1, space="PSUM"))
                for half in range(2):
                    gsl = slice(half * (G3 // 2), (half + 1) * (G3 // 2))
                    gh_t = gh_psum.tile([BC, G3 // 2], F32, tag="gh",
                                        name="gh_t")
                    for c in range(DCH):
                        w_t = wg_pool.tile([128, G3], BF16, tag="wg",
                                           name="whh_t")
                        nc.sync.dma_start(
                            out=w_t[:],
                            in_=io["whh_t"].ap()[c * 128:(c + 1) * 128, :])
                        lhs = hid_t_sb[:, c * BC:(c + 1) * BC]
                        for j in range(G3 // 2 // 512):
                            nc.tensor.matmul(
                                gh_t[:BC, j * 512:(j + 1) * 512], lhs,
                                w_t[:, half * (G3 // 2) + j * 512:
                                    half * (G3 // 2) + (j + 1) * 512],
                                start=(c == 0), stop=False)
                    for j in range(G3 // 2 // 512):
                        nc.tensor.matmul(
                            gh_t[:BC, j * 512:(j + 1) * 512],
                            ones_row[:1, :BC],
                            bhh_sb[:1, half * (G3 // 2) + j * 512:
                                   half * (G3 // 2) + (j + 1) * 512],
                            start=False, stop=(j == G3 // 2 // 512 - 1))
                    nc.scalar.activation(gh_all[:, gsl], gh_t[:BC, :],
                                         AF.Copy)

            proj_e_quarter(1)

            with ExitStack() as gctx:
                gi_psum = gctx.enter_context(
                    tc.tile_pool(name="gi_psum", bufs=1, space="PSUM"))
                for half in range(2):
                    gsl = slice(half * (G3 // 2), (half + 1) * (G3 // 2))
                    gi_t = gi_psum.tile([BC, G3 // 2], F32, tag="gie",
                                        name="gi_t")
                    for c in range(MCH):
                        w_t = wg_pool.tile([128, G3], BF16, tag="wg",
                                           name="wih_t")
                        nc.sync.dma_start(
                            out=w_t[:],
                            in_=io["wih_t"].ap()[c * 128:(c + 1) * 128, :])
                        lhs = emb_sb[:, c * BC:(c + 1) * BC]
                        for j in range(G3 // 2 // 512):
                            nc.tensor.matmul(
                                gi_t[:BC, j * 512:(j + 1) * 512], lhs,
                                w_t[:, half * (G3 // 2) + j * 512:
                                    half * (G3 // 2) + (j + 1) * 512],
                                start=(c == 0), stop=False)
                    for j in range(G3 // 2 // 512):
                        nc.tensor.matmul(
                            gi_t[:BC, j * 512:(j + 1) * 512],
                            ones_row[:1, :BC],
                            bih_sb[:1, half * (G3 // 2) + j * 512:
                                   half * (G3 // 2) + (j + 1) * 512],
                            start=False, stop=(j == G3 // 2 // 512 - 1))
                    nc.scalar.activation(gie_all[:, gsl], gi_t[:BC, :],
                                         AF.Copy)
            # pre-combine r,z parts off the critical path
            nc.vector.tensor_tensor(ghe_rz[:], gh_all[:, :2 * D],
                                    gie_all[:, :2 * D], AL.add)

            for q in range(2, NQ):
                proj_e_quarter(q)
                if q - 2 < 5:
                    load_fcw(q - 2)

        # ---------------- softmax finish + context normalize --------------
        with ExitStack() as cctx:
            tp_psum = cctx.enter_context(
                tc.tile_pool(name="tp_psum", bufs=4, space="PSUM"))
            sum_t = tp_psum.tile([1, BC], F32, tag="tp", name="sum_t")
            nc.tensor.matmul(sum_t[:1, :], ones_col[:], expT_sb[:])
            nc.vector.reciprocal(recip_row[:1, :], sum_t[:1, :])
            rc_t = tp_psum.tile([BC, 1], F32, tag="tp", name="rc_t")
            nc.tensor.matmul(rc_t[:], recip_row[:1, :], ident_f32[:1, :1],
                             is_transpose=True)
            nc.vector.tensor_copy(recip_col[:], rc_t[:])
            at_t = tp_psum.tile([BC, S], BF16, tag="tp", name="at_t")
            nc.tensor.matmul(at_t[:], expT_sb[:], ident_bf[:, :],
                             is_transpose=True)
            nc.vector.tensor_scalar_mul(attn_sb[:], at_t[:], recip_col[:])
            nc.sync.dma_start(out=io["attn_out"].ap(), in_=attn_sb[:])

            ctx_raw = singles.tile([BC, E2], BF16)
            nc.sync.dma_start(out=ctx_raw[:], in_=ctx_dram[:])
            nc.vector.tensor_scalar_mul(ctx_bf[:], ctx_raw[:], recip_col[:])
            for ec in range(ECH):
                ct_t = tp_psum.tile([128, BC], BF16, tag="tp", name="ct_t")
                nc.tensor.matmul(
                    ct_t[:], ctx_bf[:, ec * 128:(ec + 1) * 128],
                    ident_bf[:BC, :BC], is_transpose=True)
                nc.vector.tensor_copy(ctx_t_sb[:, ec * BC:(ec + 1) * BC],
                                      ct_t[:])

        # ---------------- GRU tail ----------------------------------------
        with ExitStack() as gctx:
            gic_psum = gctx.enter_context(
                tc.tile_pool(name="gic_psum", bufs=1, space="PSUM"))
            gru = gctx.enter_context(tc.tile_pool(name="gru", bufs=1))
            gic_t = gic_psum.tile([BC, G3], F32)
            for c in range(ECH):
                w_t = wg_pool.tile([128, G3], BF16, tag="wg", name="wih2_t")
                nc.sync.dma_start(
                    out=w_t[:],
                    in_=io["wih_t"].ap()[(MCH + c) * 128:
                                         (MCH + c + 1) * 128, :])
                lhs = ctx_t_sb[:, c * BC:(c + 1) * BC]
                for j in range(G3 // 512):
                    nc.tensor.matmul(
                        gic_t[:BC, j * 512:(j + 1) * 512], lhs,
                        w_t[:, j * 512:(j + 1) * 512],
                        start=(c == 0), stop=(c == ECH - 1))
            t_rz = gru.tile([BC, 2 * D], F32)
            r_sb = gru.tile([BC, D], F32)
            z_sb = gru.tile([BC, D], F32)
            n_sb = gru.tile([BC, D], F32)
            tmp = gru.tile([BC, D], F32)
            nc.vector.tensor_tensor(t_rz[:], gic_t[:BC, :2 * D], ghe_rz[:],
                                    AL.add)
            nc.scalar.activation(r_sb[:], t_rz[:, :D], AF.Sigmoid)
            nc.scalar.activation(z_sb[:], t_rz[:, D:], AF.Sigmoid)
            nc.vector.tensor_tensor(tmp[:], gic_t[:BC, 2 * D:],
                                    gie_all[:, 2 * D:], AL.add)
            nc.vector.tensor_tensor(n_sb[:], r_sb[:], gh_all[:, 2 * D:],
                                    AL.mult)
            nc.vector.tensor_tensor(tmp[:], tmp[:], n_sb[:], AL.add)
            nc.scalar.activation(n_sb[:], tmp[:], AF.Tanh)
            nc.vector.tensor_tensor(tmp[:], hid_f_sb[:], n_sb[:], AL.subtract)
            nc.vector.tensor_tensor(tmp[:], z_sb[:], tmp[:], AL.mult)
            nc.vector.tensor_tensor(hnew_sb[:], n_sb[:], tmp[:], AL.add)
            nc.sync.dma_start(out=io["hnew_out"].ap(), in_=hnew_sb[:])

        # ---------------- allgather + fc ----------------------------------
        with ExitStack() as dctx:
            ht_psum = dctx.enter_context(
                tc.tile_pool(name="ht_psum", bufs=2, space="PSUM"))
            for ec in range(DCH):
                ht_t = ht_psum.tile([128, BC], F32, tag="ht", name="ht_t")
                nc.tensor.matmul(
                    ht_t[:], hnew_sb[:, ec * 128:(ec + 1) * 128],
                    ident_f32[:BC, :BC], is_transpose=True)
                nc.vector.tensor_copy(hnt_sb[:, ec * BC:(ec + 1) * BC],
                                      ht_t[:])
            dram = dctx.enter_context(
                tc.tile_pool(name="dram", bufs=1, space="DRAM"))
            hn_loc = dram.tile([D, BC], BF16)
            hn_all = dram.tile([NCORES * D, BC], BF16, addr_space="Shared")
            nc.sync.dma_start(
                out=hn_loc[:].rearrange("(c p) b -> p c b", p=128),
                in_=hnt_sb[:].rearrange("p (c b) -> p c b", c=DCH))
            nc.gpsimd.collective_compute(
                "AllGather", mybir.AluOpType.bypass,
                replica_groups=[list(range(NCORES))],
                ins=[hn_loc[:].opt()], outs=[hn_all[:].opt()])
            for dc in range(DCH):
                nc.sync.dma_start(
                    out=hna_sb[:, dc * B:(dc + 1) * B].rearrange(
                        "p (c b) -> p c b", c=NCORES),
                    in_=hn_all[:].rearrange(
                        "(c k p) b -> k p c b", c=NCORES, p=128)[dc])

            fc_psum = dctx.enter_context(
                tc.tile_pool(name="fc_psum", bufs=4, space="PSUM"))
            pred_pool = dctx.enter_context(
                tc.tile_pool(name="pred_pool", bufs=3))
            for vq in range(NVQ):
                if vq not in fcw_tiles:
                    load_fcw(vq)
                fq = fcw_tiles.pop(vq)
                for bt in range(B // 128):
                    fc_t = fc_psum.tile([128, VQ], F32, tag="fc", name="fc_t")
                    for dc in range(DCH):
                        nc.tensor.matmul(
                            fc_t[:],
                            hna_sb[:, dc * B + bt * 128:
                                   dc * B + (bt + 1) * 128],
                            fq[:, dc * VQ:(dc + 1) * VQ],
                            start=(dc == 0), stop=False)
                    nc.tensor.matmul(
                        fc_t[:], ones_row[:1, :],
                        fcb_sb[:1, vq * VQ:(vq + 1) * VQ],
                        start=False, stop=True)
                    pred_t = pred_pool.tile([128, VQ], F32, tag="pred",
                                            name="pred_t")
                    nc.scalar.activation(pred_t[:], fc_t[:], AF.Copy)
                    nc.sync.dma_start(
                        out=io["pred_out"].ap()[bt * 128:(bt + 1) * 128,
                                                vq * VQ:(vq + 1) * VQ],
                        in_=pred_t[:])


def _build_program() -> bass.Bass:
    nc = bacc.Bacc("TRN2", target_bir_lowering=False, debug=False,
                   num_devices=NCORES)
    io = {}
    io["emb_t"] = nc.dram_tensor("emb_t", [M, BC], BF16, kind="ExternalInput")
    io["hid_t"] = nc.dram_tensor("hid_t", [D, BC], BF16, kind="ExternalInput")
    io["hid_f"] = nc.dram_tensor("hid_f", [BC, D], F32, kind="ExternalInput")
    io["enc_t"] = nc.dram_tensor("enc_t", [E2, BS], BF16, kind="ExternalInput")
    io["enc_n"] = nc.dram_tensor("enc_n", [BS, E2], BF16, kind="ExternalInput")
    io["wa_t"] = nc.dram_tensor("wa_t", [D, D], BF16, kind="ExternalInput")
    io["ua_t"] = nc.dram_tensor("ua_t", [E2, D], BF16, kind="ExternalInput")
    io["bab"] = nc.dram_tensor("bab", [1, D], BF16, kind="ExternalInput")
    io["v_w"] = nc.dram_tensor("v_w", [D], BF16, kind="ExternalInput")
    io["wih_t"] = nc.dram_tensor("wih_t", [FI, G3], BF16, kind="ExternalInput")
    io["whh_t"] = nc.dram_tensor("whh_t", [D, G3], BF16, kind="ExternalInput")
    io["bih"] = nc.dram_tensor("bih", [1, G3], BF16, kind="ExternalInput")
    io["bhh"] = nc.dram_tensor("bhh", [1, G3], BF16, kind="ExternalInput")
    io["fcw_t"] = nc.dram_tensor("fcw_t", [D, VC], BF16, kind="ExternalInput")
    io["fcb"] = nc.dram_tensor("fcb", [1, VC], BF16, kind="ExternalInput")
    if os.environ.get("DECODER_KERNEL_DEBUG"):
        io["dbg_hn"] = nc.dram_tensor("dbg_hn", [NCORES * D, BC], BF16,
                                      kind="ExternalOutput")
    io["pred_out"] = nc.dram_tensor("pred_out", [B, VC], F32,
                                    kind="ExternalOutput")
    io["hnew_out"] = nc.dram_tensor("hnew_out", [BC, D], F32,
                                    kind="ExternalOutput")
    io["attn_out"] = nc.dram_tensor("attn_out", [BC, S], F32,
                                    kind="ExternalOutput")

    with tile.TileContext(nc) as tc:
        _emit(nc, tc, io)
    nc.compile()
    return nc


_PROGRAM_CACHE = None


def _get_program():
    global _PROGRAM_CACHE
    if _PROGRAM_CACHE is None:
        _PROGRAM_CACHE = _build_program()
    return _PROGRAM_CACHE


def _prep_in_maps(x, hidden, encoder_outputs, emb, Wa, ba, Ua, bUa, v,
                  W_ih, W_hh, b_ih, b_hh, fc_W, fc_b):
    f32 = np.float32
    x = np.asarray(x)
    hidden = np.asarray(hidden, f32)
    enc = np.asarray(encoder_outputs, f32)
    emb = np.asarray(emb, f32)
    embedded = emb[x]                              # [B, M] host gather
    bab = (np.asarray(ba, f32) + np.asarray(bUa, f32))
    bab = bab.astype(NPBF16).reshape(1, D)
    wa_t = np.ascontiguousarray(np.asarray(Wa, f32).T).astype(NPBF16)
    ua_t = np.ascontiguousarray(np.asarray(Ua, f32).T).astype(NPBF16)
    wih_t = np.ascontiguousarray(np.asarray(W_ih, f32).T).astype(NPBF16)
    whh_t = np.ascontiguousarray(np.asarray(W_hh, f32).T).astype(NPBF16)
    v_bf = np.asarray(v, f32).astype(NPBF16)
    bih = np.asarray(b_ih, f32).astype(NPBF16).reshape(1, G3)
    bhh = np.asarray(b_hh, f32).astype(NPBF16).reshape(1, G3)
    fc_W = np.asarray(fc_W, f32)
    fc_b = np.asarray(fc_b, f32)

    in_maps = []
    for c in range(NCORES):
        bs_ = slice(c * BC, (c + 1) * BC)
        vs_ = slice(c * VC, (c + 1) * VC)
        enc_c = enc[bs_]                                   # [32, 128, 1024]
        in_maps.append({
            "emb_t": np.ascontiguousarray(embedded[bs_].T).astype(NPBF16),
            "hid_t": np.ascontiguousarray(hidden[bs_].T).astype(NPBF16),
            "hid_f": np.ascontiguousarray(hidden[bs_]),
            "enc_t": np.ascontiguousarray(
                enc_c.transpose(2, 0, 1).reshape(E2, BS)).astype(NPBF16),
            "enc_n": np.ascontiguousarray(
                enc_c.reshape(BS, E2)).astype(NPBF16),
            "wa_t": wa_t,
            "ua_t": ua_t,
            "bab": bab,
            "v_w": v_bf,
            "wih_t": wih_t,
            "whh_t": whh_t,
            "bih": bih,
            "bhh": bhh,
            "fcw_t": np.ascontiguousarray(fc_W[vs_].T).astype(NPBF16),
            "fcb": fc_b[vs_].astype(NPBF16).reshape(1, VC),
        })
    return in_maps


def _install_ntff_hook():
    """The agent image's antenv lacks axon_hooks; synthesize it so
    trace=True can capture NTFF profiles through libaxon_pjrt.so."""
    import types

    if "antenv.axon_hooks" in sys.modules:
        return
    import antenv

    mod = types.ModuleType("antenv.axon_hooks")
    state = {"hook": None}
    mod.set_axon_ntff_profile_hook = lambda h: state.__setitem__("hook", h)
    mod.get_axon_ntff_profile_hook = lambda: state["hook"]
    sys.modules["antenv.axon_hooks"] = mod
    antenv.axon_hooks = mod
    try:
        from trn_agent_boot.trn_boot import _ntff_profile_via_ctypes
        mod.set_axon_ntff_profile_hook(
            _ntff_profile_via_ctypes("/opt/axon/libaxon_pjrt.so"))
    except Exception as e:  # degrade to no tracing
        print(f"ntff hook install failed: {e}", file=sys.stderr)
    # no S3 in this container; keep artifacts local
    import concourse.bass_utils as _bu
    _bu.upload_artifacts = lambda tmpdir: tmpdir


def _run(in_maps, trace=False):
    nc = _get_program()
    if trace:
        _install_ntff_hook()
    return run_bass_kernel_spmd(nc, in_maps, core_ids=list(range(NCORES)),
                                trace=trace)


def kernel(x, hidden, encoder_outputs, emb, Wa, ba, Ua, bUa, v,
           W_ih, W_hh, b_ih, b_hh, fc_W, fc_b, _trace=False,
           _want_results=False):
    in_maps = _prep_in_maps(x, hidden, encoder_outputs, emb, Wa, ba, Ua, bUa,
                            v, W_ih, W_hh, b_ih, b_hh, fc_W, fc_b)
    res = _run(in_maps, trace=_trace)
    pred = np.concatenate([res.results[c]["pred_out"] for c in range(NCORES)],
                          axis=1)
    h_new = np.concatenate([res.results[c]["hnew_out"] for c in range(NCORES)],
                           axis=0)
    attn_w = np.concatenate([res.results[c]["attn_out"] for c in range(NCORES)],
                            axis=0)
    out = (np.asarray(pred, np.float32), np.asarray(h_new, np.float32),
           np.asarray(attn_w, np.float32))
    if _want_results:
        return out, res
    return out
